# revision 1
# baseline (speedup 1.0000x reference)
"""Trainium2 Bass kernel for the didgeridoo (conical bore) input-impedance model.

Math (matches the reference): for each integer frequency f in [fmin, fmax),
chain-multiply 128 per-slice lossy transmission-line 2x2 complex matrices
    T_n = [[ch_n, Z0_n*sh_n], [sh_n/Z0_n, ch_n]],   gamma_n = (alpha_n + i*k)*dL
then Ze = (A*ZL + B)/(C*ZL + D) against the unflanged-open-end radiation
impedance ZL, output |Ze|.

Kernel strategy (per the sharding hint): frequencies are sharded 8 ways
across cores (47 per core, padded), each core puts its frequencies on the
SBUF partition axis and the 128 bore slices on the free axis. The ordered
matrix product is a binary tree (7 levels) over planes packed re|im x
(A,B,C,D) in one tile: per level 8 strided tensor-multiplies (split
Vector/GPSIMD) write a term-interleaved tile and ONE tensor_reduce(add)
over the innermost 4 yields the next level; real-part sign flips come
from an ACT-negated imag half. The radiation load ZL is folded into the
last slice matrix so the tail is just |A|/|C|. cosh/sinh/cos/sin
arguments are all < 0.07 here, so fp32-exact Taylor polynomials replace
transcendentals.
"""
import math
from contextlib import ExitStack

import numpy as np

import concourse.bass as bass
import concourse.bacc as bacc
import concourse.tile as tile
from concourse import mybir
from concourse.bass_utils import run_bass_kernel_spmd

RHO = 1.2929
C_SOUND = 343.37
N_SUB = 128
N_CORES = 8
D0 = 32.0

F32 = mybir.dt.float32
MULT = mybir.AluOpType.mult
ADD = mybir.AluOpType.add
SUB = mybir.AluOpType.subtract
IDENT = mybir.ActivationFunctionType.Identity
COPY = mybir.ActivationFunctionType.Copy
SQUARE = mybir.ActivationFunctionType.Square
SQRT = mybir.ActivationFunctionType.Sqrt


def _sel(tsb, part, base_entry, entry_step, n, m, odd):
    """Strided selection AP over a packed [P, 4*n] plane tile.

    Pattern: entries (e, e, e+s, e+s) x (left|right of each adjacent pair).
    dims: [[entry_step*n, 2], [0, 2], [2, m]] starting at base_entry*n (+1 if odd).
    """
    off = base_entry * n + (1 if odd else 0)
    return bass.AP(tsb, off, [part, [entry_step * n, 2], [0, 2], [2, m]])


def _rsel(tsb, part, base_entry, entry_step, n, m, odd):
    """Right-operand pattern: entries (e, e+s, e, e+s)."""
    off = base_entry * n + (1 if odd else 0)
    return bass.AP(tsb, off, [part, [0, 2], [entry_step * n, 2], [2, m]])


def _outv(tsb, part, m):
    """Contiguous [P, 2, 2, m] view of a packed [P, 4*m] tile."""
    return bass.AP(tsb, 0, [part, [2 * m, 2], [m, 2], [1, m]])


def _emit_body(nc, tc, pool, P, xd, outd):
    """Emit one full evaluation: DMA in -> compute -> DMA out.

    Unified complex plane tile [re(4n) | im(4n)]; per tree level 8 plain
    mults (Vector/GPSIMD split) write a term-interleaved tile and ONE
    tensor_reduce(add, innermost-4) produces both re and im of the next
    level; the real-part negations come from an ACT-built negated-imag half.
    Prep chain runs on Vector in 2x tensor_scalar mode; the radiation load
    ZL is folded into the last slice matrix (E = [[ZL,0],[1,0]]).
    """
    N = N_SUB

    def T(w, tag):
        return pool.tile([P, w], F32, name=tag, tag=tag)

    V, G, S = nc.vector, nc.gpsimd, nc.scalar

    # prefetch the sqrt_and_friends activation table before the input arrives
    warm = T(1, "warm")
    S.activation(warm[:], nc.const_aps.aps[(F32, 1.0)][:P], SQRT)

    x_sb = T(4 + N, "x")
    nc.sync.dma_start(out=x_sb[:, 0:4], in_=xd.ap()[:, 0:4])
    nc.sync.dma_start(out=x_sb[:, 4:4 + N], in_=xd.ap()[:, 4:4 + N])
    f = x_sb[:, 0:1]
    sqf = x_sb[:, 1:2]
    ln = x_sb[:, 2:3]
    d1 = x_sb[:, 3:4]
    tg = x_sb[:, 4:4 + N]

    # --- prep: [P,1] scalars on ACT, [P,N] grids on Vector (2x ts mode) ---
    dL = T(1, "dL")
    S.activation(dL[:], ln, COPY, scale=10.0 / 1000.0 / N_SUB)
    y = T(1, "y")
    V.scalar_tensor_tensor(y[:], f, 2.0 * math.pi / C_SOUND, dL[:], MULT, MULT)
    s_ = T(1, "s_")
    V.scalar_tensor_tensor(s_[:], sqf, 3e-5, dL[:], MULT, MULT)
    dd = T(1, "dd")
    S.activation(dd[:], d1, IDENT, scale=1.0 / 2000.0, bias=-D0 / 2000.0)
    r = T(N, "r")
    V.tensor_scalar(r[:], tg, dd[:], D0 / 2000.0, MULT, ADD)
    rinv = T(N, "rinv")
    V.reciprocal(rinv[:], r[:])
    xg = T(N, "xg")
    V.tensor_scalar(xg[:], rinv[:], s_[:], None, MULT)
    x2 = T(N, "x2")
    V.tensor_mul(x2[:], xg[:], xg[:])
    chx = T(N, "chx")
    V.tensor_scalar(chx[:], x2[:], 0.5, 1.0, MULT, ADD)
    w6 = T(N, "w6")
    V.tensor_scalar(w6[:], x2[:], 1.0 / 6.0, 1.0, MULT, ADD)
    shx = T(N, "shx")
    V.tensor_mul(shx[:], xg[:], w6[:])
    y2 = T(1, "y2")
    S.activation(y2[:], y[:], SQUARE)
    cyh = T(1, "cyh")
    S.activation(cyh[:], y2[:], IDENT, scale=1.0 / 24.0, bias=-0.5)
    cosy = T(1, "cosy")
    S.activation(cosy[:], cyh[:], IDENT, scale=y2[:], bias=1.0)
    syh = T(1, "syh")
    S.activation(syh[:], y2[:], IDENT, scale=1.0 / 120.0, bias=-1.0 / 6.0)
    syw = T(1, "syw")
    S.activation(syw[:], syh[:], IDENT, scale=y2[:], bias=1.0)
    siny = T(1, "siny")
    S.activation(siny[:], syw[:], COPY, scale=y[:])
    nsiny = T(1, "nsiny")
    S.activation(nsiny[:], siny[:], COPY, scale=-1.0)  # -siny
    z0 = T(N, "z0")
    V.scalar_tensor_tensor(z0[:], rinv[:], RHO * C_SOUND / math.pi, rinv[:], MULT, MULT)
    z0i = T(N, "z0i")
    V.scalar_tensor_tensor(z0i[:], r[:], math.pi / (RHO * C_SOUND), r[:], MULT, MULT)
    shc = T(N, "shc")
    V.tensor_scalar(shc[:], shx[:], cosy[:], None, MULT)
    chs = T(N, "chs")
    V.tensor_scalar(chs[:], chx[:], siny[:], None, MULT)

    # radiation impedance ZL [P,1]
    r_end = T(1, "r_end")
    S.activation(r_end[:], d1, COPY, scale=1.0 / 2000.0)
    rinv_e = T(1, "rinv_e")
    V.reciprocal(rinv_e[:], r_end[:])
    kr = T(1, "kr")
    V.scalar_tensor_tensor(kr[:], f, 2.0 * math.pi / C_SOUND, r_end[:], MULT, MULT)
    z0e = T(1, "z0e")
    V.scalar_tensor_tensor(z0e[:], rinv_e[:], RHO * C_SOUND / math.pi, rinv_e[:], MULT, MULT)
    kr2 = T(1, "kr2")
    S.activation(kr2[:], kr[:], SQUARE)
    zlre = T(1, "zlre")
    V.scalar_tensor_tensor(zlre[:], kr2[:], 0.25, z0e[:], MULT, MULT)
    zlim = T(1, "zlim")
    V.scalar_tensor_tensor(zlim[:], kr[:], 0.61, z0e[:], MULT, MULT)

    # --- level-0 planes: unified [P, re(A,B,C,D) | im(A,B,C,D)] ---
    # layout: re entries at 0,N,2N,3N ; im at 4N..7N (D = A at level 0).
    # Slices 0..126 come from the bulk builds; slice 127 (the E-fold column,
    # T'127 = T127 @ [[ZL,0],[1,0]]) is computed straight from prep values so
    # the fold runs CONCURRENTLY with the bulk plane builds.
    M = N - 1
    lc = N - 1
    pc = T(8 * N, "pc0")
    S.activation(pc[:, 0:M], chx[:, 0:M], COPY, scale=cosy[:])           # A_re
    S.activation(pc[:, 4 * N:4 * N + M], shx[:, 0:M], COPY, scale=siny[:])   # A_im
    V.tensor_mul(pc[:, N:N + M], z0[:, 0:M], shc[:, 0:M])                # B_re
    V.tensor_mul(pc[:, 5 * N:5 * N + M], z0[:, 0:M], chs[:, 0:M])        # B_im
    G.tensor_mul(pc[:, 2 * N:2 * N + M], z0i[:, 0:M], shc[:, 0:M])       # C_re
    G.tensor_mul(pc[:, 6 * N:6 * N + M], z0i[:, 0:M], chs[:, 0:M])       # C_im
    S.activation(pc[:, 3 * N:3 * N + M], chx[:, 0:M], COPY, scale=cosy[:])   # D_re
    S.activation(pc[:, 7 * N:7 * N + M], shx[:, 0:M], COPY, scale=siny[:])   # D_im

    # negated imag half for level-1 real-part products (slices 0..126)
    ng = T(4 * N, "ng0")
    S.activation(ng[:, 0:M], shx[:, 0:M], COPY, scale=nsiny[:])          # -A_im
    S.activation(ng[:, N:N + M], pc[:, 5 * N:5 * N + M], COPY, scale=-1.0)   # -B_im
    S.activation(ng[:, 2 * N:2 * N + M], pc[:, 6 * N:6 * N + M], COPY, scale=-1.0)  # -C_im
    S.activation(ng[:, 3 * N:3 * N + M], shx[:, 0:M], COPY, scale=nsiny[:])  # -D_im

    # folded column 127, from prep values only (parallel with bulk builds):
    # T127 entries, then A' = A*ZL + B ; C' = C*ZL + A ; B' = D' = 0
    ch7 = chx[:, lc:lc + 1]
    sh7 = shx[:, lc:lc + 1]
    ar0 = T(1, "ar0")
    V.tensor_scalar(ar0[:], ch7, cosy[:], None, MULT)        # A127 re
    ai0 = T(1, "ai0")
    V.tensor_scalar(ai0[:], sh7, siny[:], None, MULT)        # A127 im
    sc0 = T(1, "sc0")
    V.tensor_scalar(sc0[:], sh7, cosy[:], None, MULT)        # sh*cosy
    ci0 = T(1, "ci0")
    V.tensor_scalar(ci0[:], ch7, siny[:], None, MULT)        # ch*siny
    br0 = T(1, "br0")
    G.tensor_mul(br0[:], z0[:, lc:lc + 1], sc0[:])           # B127 re
    bi0 = T(1, "bi0")
    G.tensor_mul(bi0[:], z0[:, lc:lc + 1], ci0[:])           # B127 im
    cr0 = T(1, "cr0")
    G.tensor_mul(cr0[:], z0i[:, lc:lc + 1], sc0[:])          # C127 re
    cib = T(1, "cib")
    G.tensor_mul(cib[:], z0i[:, lc:lc + 1], ci0[:])          # C127 im
    e1 = T(1, "e1")
    V.tensor_scalar(e1[:], ar0[:], zlre[:], br0[:], MULT, ADD)   # Are*ZLre + Bre
    e2 = T(1, "e2")
    V.tensor_scalar(e2[:], ai0[:], zlim[:], None, MULT)          # Aim*ZLim
    e3 = T(1, "e3")
    V.tensor_scalar(e3[:], ar0[:], zlim[:], bi0[:], MULT, ADD)   # Are*ZLim + Bim
    e4 = T(1, "e4")
    V.tensor_scalar(e4[:], ai0[:], zlre[:], None, MULT)          # Aim*ZLre
    g1 = T(1, "g1")
    V.tensor_scalar(g1[:], cr0[:], zlre[:], ar0[:], MULT, ADD)   # Cre*ZLre + Dre(=Are)
    g2 = T(1, "g2")
    V.tensor_scalar(g2[:], cib[:], zlim[:], None, MULT)
    g3 = T(1, "g3")
    V.tensor_scalar(g3[:], cr0[:], zlim[:], ai0[:], MULT, ADD)
    g4 = T(1, "g4")
    V.tensor_scalar(g4[:], cib[:], zlre[:], None, MULT)
    G.tensor_sub(pc[:, lc:lc + 1], e1[:], e2[:])                 # A'127 re
    G.tensor_add(pc[:, 4 * N + lc:4 * N + lc + 1], e3[:], e4[:])  # A'127 im
    G.tensor_sub(pc[:, 2 * N + lc:2 * N + lc + 1], g1[:], g2[:])  # C'127 re
    G.tensor_add(pc[:, 6 * N + lc:6 * N + lc + 1], g3[:], g4[:])  # C'127 im
    # B'127 = D'127 = 0 and ng column 127 (from const-0; cols never written
    # by the bulk builds, so fill fresh rather than in-place scaling)
    zero_ap = nc.const_aps.aps[(F32, 0.0)][:P]
    S.activation(pc[:, N + lc:N + lc + 1], zero_ap, COPY)         # B'127 re
    S.activation(pc[:, 5 * N + lc:5 * N + lc + 1], zero_ap, COPY)  # B'127 im
    S.activation(pc[:, 3 * N + lc:3 * N + lc + 1], zero_ap, COPY)  # D'127 re
    S.activation(pc[:, 7 * N + lc:7 * N + lc + 1], zero_ap, COPY)  # D'127 im
    S.activation(ng[:, lc:lc + 1], pc[:, 4 * N + lc:4 * N + lc + 1], COPY, scale=-1.0)
    S.activation(ng[:, N + lc:N + lc + 1], zero_ap, COPY)
    S.activation(ng[:, 2 * N + lc:2 * N + lc + 1], pc[:, 6 * N + lc:6 * N + lc + 1], COPY, scale=-1.0)
    S.activation(ng[:, 3 * N + lc:3 * N + lc + 1], zero_ap, COPY)

    # --- binary tree: per level 8 mults + 1 fused reduce ---
    n = N
    lvl = 0
    im_off = 4 * N  # offset of the imag half in the current plane tile
    ng_t = ng
    while n > 1:
        m = n // 2
        lvl += 1
        h = pc[:].tensor
        pd = [pc[:].ap[0][0], P]
        hn = ng_t[:].tensor
        pdn = [ng_t[:].ap[0][0], P]

        l1r = bass.AP(h, 0, [pd, [2 * n, 2], [0, 2], [2, m]])
        l1i = bass.AP(h, im_off, [pd, [2 * n, 2], [0, 2], [2, m]])
        l1n = bass.AP(hn, 0, [pdn, [2 * n, 2], [0, 2], [2, m]])
        r1r = bass.AP(h, 1, [pd, [0, 2], [n, 2], [2, m]])
        r1i = bass.AP(h, im_off + 1, [pd, [0, 2], [n, 2], [2, m]])
        l2r = bass.AP(h, n, [pd, [2 * n, 2], [0, 2], [2, m]])
        l2i = bass.AP(h, im_off + n, [pd, [2 * n, 2], [0, 2], [2, m]])
        l2n = bass.AP(hn, n, [pdn, [2 * n, 2], [0, 2], [2, m]])
        r2r = bass.AP(h, 2 * n + 1, [pd, [0, 2], [n, 2], [2, m]])
        r2i = bass.AP(h, im_off + 2 * n + 1, [pd, [0, 2], [n, 2], [2, m]])

        # term-interleaved products: re terms at c=0, im at c=1
        # element (c, e, p, t) at c*16m + 4*(e*m+p) + t
        u = T(32 * m, f"u{lvl}")
        uh = u[:].tensor
        upd = [u[:].ap[0][0], P]

        def tm(c, t):
            return bass.AP(uh, c * 16 * m + t, [upd, [8 * m, 2], [4 * m, 2], [4, m]])

        # real part: t0=Lre1*Rre1 t1=Lre2*Rre2 t2=(-Lim1)*Rim1 t3=(-Lim2)*Rim2
        V.tensor_tensor(tm(0, 0), l1r, r1r, MULT)
        V.tensor_tensor(tm(0, 1), l2r, r2r, MULT)
        # imag part: Lre*Rim + Lim*Rre; G long pole at small levels -> shift
        # one imag mult to Vector there
        (V if n <= 16 else G).tensor_tensor(tm(1, 0), l1r, r1i, MULT)
        G.tensor_tensor(tm(1, 1), l2r, r2i, MULT)
        G.tensor_tensor(tm(1, 2), l1i, r1r, MULT)
        G.tensor_tensor(tm(1, 3), l2i, r2r, MULT)
        # negim-dependent last (off Vector at big levels so the reduces
        # don't wait on the ACT-negate hop)
        (G if n >= 64 else V).tensor_tensor(tm(0, 2), l1n, r1i, MULT)
        G.tensor_tensor(tm(0, 3), l2n, r2i, MULT)

        q = T(8 * m, f"pc{lvl}")
        rin_r = bass.AP(uh, 0, [upd, [4, 4 * m], [1, 4]])
        rin_i = bass.AP(uh, 16 * m, [upd, [4, 4 * m], [1, 4]])
        V.tensor_reduce(q[:, 0:4 * m], rin_r, mybir.AxisListType.X, ADD)
        V.tensor_reduce(q[:, 4 * m:8 * m], rin_i, mybir.AxisListType.X, ADD)

        if m > 1:
            ngn = T(4 * m, f"ng{lvl}")
            S.activation(ngn[:], q[:, 4 * m:8 * m], COPY, scale=-1.0)
            ng_t = ngn
        pc = q
        im_off = 4 * m
        n = m

    # --- final: num = A (entries 0re / 4im), den = C (2re / 6im) ---
    are, aim = pc[:, 0:1], pc[:, 4:5]
    cre, cim = pc[:, 2:3], pc[:, 6:7]
    n2a = T(1, "n2a")
    S.activation(n2a[:], are, SQUARE)
    n2b = T(1, "n2b")
    S.activation(n2b[:], aim, SQUARE)
    n2 = T(1, "n2")
    V.tensor_add(n2[:], n2a[:], n2b[:])
    d2a = T(1, "d2a")
    S.activation(d2a[:], cre, SQUARE)
    d2b = T(1, "d2b")
    S.activation(d2b[:], cim, SQUARE)
    d2 = T(1, "d2")
    G.tensor_add(d2[:], d2a[:], d2b[:])
    d2r = T(1, "d2r")
    V.reciprocal(d2r[:], d2[:])
    rat = T(1, "rat")
    V.tensor_mul(rat[:], n2[:], d2r[:])
    res = T(1, "res")
    S.activation(res[:], rat[:], SQRT)

    nc.sync.dma_start(out=outd.ap(), in_=res[:])


def build_program(fpc, loop_iters=None):
    """Build the SPMD Bass program; every core runs it on its own 47 freqs.

    loop_iters: if set, wrap the body in a hardware For_i loop (used only by
    the timing harness to amortize dispatch overhead)."""
    nc = bacc.Bacc("TRN2", target_bir_lowering=False, debug=False)
    P = fpc
    N = N_SUB

    # activation-bias constants beyond the built-in 0.0/1.0
    for cv in (-D0 / 2000.0, D0 / 2000.0, -0.5, -1.0 / 6.0):
        th = nc.alloc_sbuf_tensor(f"cst{cv}", [128, 1], F32)
        nc.gpsimd.memset(th.ap(), cv)
        nc.const_aps.aps[(F32, cv)] = th.ap()
    nc.all_engine_barrier()

    xd = nc.dram_tensor("x", [P, 4 + N], F32, kind="ExternalInput")
    outd = nc.dram_tensor("out", [P, 1], F32, kind="ExternalOutput")

    with tile.TileContext(nc) as tc, ExitStack() as ctx:
        pool = ctx.enter_context(tc.tile_pool(name="p", bufs=1))
        if loop_iters is None:
            _emit_body(nc, tc, pool, P, xd, outd)
        else:
            with tc.For_i(0, loop_iters, 1):
                _emit_body(nc, tc, pool, P, xd, outd)

    nc.compile()
    return nc


_PROGRAM_CACHE = {}


def _get_program(fpc):
    if fpc not in _PROGRAM_CACHE:
        _PROGRAM_CACHE[fpc] = build_program(fpc)
    return _PROGRAM_CACHE[fpc]


def make_inputs(length, d1, fmin, fmax, fpc):
    """Host-side shard prep: pack [f | length | d1 | t] per core. No math on
    device-owned values beyond replication."""
    F = fmax - fmin
    f_full = np.arange(fmin, fmax, dtype=np.float32)
    f_pad = np.concatenate([f_full, np.full(N_CORES * fpc - F, float(fmin), np.float32)])
    t = ((np.arange(N_SUB, dtype=np.float32) + 0.5) / N_SUB)
    in_maps = []
    for c in range(N_CORES):
        X = np.empty((fpc, 4 + N_SUB), dtype=np.float32)
        X[:, 0] = f_pad[c * fpc:(c + 1) * fpc]
        X[:, 1] = np.sqrt(f_pad[c * fpc:(c + 1) * fpc])
        X[:, 2] = np.float32(length[0])
        X[:, 3] = np.float32(d1[0])
        X[:, 4:] = t[None, :]
        in_maps.append({"x": X})
    return in_maps


def kernel(length, d1, fmin, fmax):
    length = np.asarray(length, dtype=np.float32)
    d1 = np.asarray(d1, dtype=np.float32)
    fmin = int(fmin)
    fmax = int(fmax)
    F = fmax - fmin
    fpc = (F + N_CORES - 1) // N_CORES
    nc = _get_program(fpc)
    in_maps = make_inputs(length, d1, fmin, fmax, fpc)
    res = run_bass_kernel_spmd(nc, in_maps, list(range(N_CORES)))
    outs = [res.results[c]["out"].reshape(-1) for c in range(N_CORES)]
    return np.concatenate(outs)[:F].astype(np.float32)

